# revision 1
# baseline (speedup 1.0000x reference)
"""Trainium2 Bass kernel for LowDimQKMultiHeadAttention.

Problem shapes (hardcoded): B=4, Tq=Tk=2048, D=1024, H=8 heads,
QK_DIM=256 (32 per head), head_v=128, fp32 I/O.

Sharding over 8 NeuronCores: core c handles batch b=c//2 and head-group
g=c%2 (4 heads = qk cols [128g,128g+128), v cols [512g, 512g+512)).
Each core is fully independent (no collectives).

Per-core algorithm:
  1. qT/kT projection (K first, then Q block 0-1, per 256-row half-block):
     DMA, transpose 128x128 tiles on PE (fp32r), stage PSUM->SBUF on DVE,
     project with Wq/Wk as stationary operand (fp32r), bias-add on the
     PSUM->SBUF copy into per-block [64, 512] tiles (2 heads per tile;
     matmul operand APs must start at partition 0/32/64). Q blocks 2-3
     are deferred into the attention interleave slots.
  2. Attention per (tq-chunk of 1024, head), software-pipelined: the PE
     emission interleaves chunk c's PV accumulation groups between chunk
     c+1's scores matmuls so ACT (exp, the bottleneck engine) never
     starves. scoresT[tk=128, tq] via fp32r K=32 N=512 matmuls, two per
     2-bank PSUM tile; one ACT exp per [128,1024] tile (bf16 out, fused
     1/sqrt(32) scale + additive key-padding-mask per-partition bias).
  3. PV with fused softmax denominator: rhs = [V_h | ones] bf16 (129
     cols); out[tq=128, 129] += attnT_tile.T @ rhs accumulated over 16
     tk-tiles in PSUM; column 128 is sum(exp). Normalize with DVE
     reciprocal + tensor_scalar_mul, DMA out.

NaN-scrub from the reference is skipped (inputs are finite, scores cannot
be NaN). Key padding mask is applied as an additive -1e30 bias.
"""

import math

import numpy as np

import concourse.bacc as bacc
import concourse.mybir as mybir
import concourse.tile as tile
from concourse.bass_utils import run_bass_kernel_spmd
from concourse.masks import make_identity

dt = mybir.dt

B = 4
T = 2048          # Tq == Tk
D = 1024
H = 8
HEAD_QK = 32
CG = 128          # qk cols per core (4 heads * 32)
VG = 512          # v cols per core (4 heads * 128)
HV = 128          # head_v
NBLK = 4          # 512-row blocks of T
NTILE = 16        # 128-row tiles of T
SCALE = 1.0 / math.sqrt(HEAD_QK)
VEXT = HV + 1     # V cols + ones column per head

_cache = {}


def _build(loop_n=1):
    nc = bacc.Bacc("TRN2", target_bir_lowering=False, debug=False, num_devices=8)

    Q = nc.declare_dram_parameter("Q", [T, D], dt.float32, isOutput=False)
    K = nc.declare_dram_parameter("K", [T, D], dt.float32, isOutput=False)
    V = nc.declare_dram_parameter("V", [T, VG], dt.float32, isOutput=False)
    Wq = nc.declare_dram_parameter("Wq", [D, CG], dt.float32, isOutput=False)
    Wk = nc.declare_dram_parameter("Wk", [D, CG], dt.float32, isOutput=False)
    bq = nc.declare_dram_parameter("bq", [CG, 1], dt.float32, isOutput=False)
    bk = nc.declare_dram_parameter("bk", [CG, 1], dt.float32, isOutput=False)
    maskb = nc.declare_dram_parameter("maskb", [128, NTILE], dt.float32,
                                      isOutput=False)
    O = nc.declare_dram_parameter("O", [T, VG], dt.float32, isOutput=True)

    f32, f32r, bf16 = dt.float32, dt.float32r, dt.bfloat16

    with tile.TileContext(nc) as tc:
        with tc.tile_pool(name="consts", bufs=1) as cp, \
             tc.tile_pool(name="sb", bufs=1) as sb, \
             tc.tile_pool(name="ps", bufs=1, space="PSUM") as ps:
            # ---- constants ----
            ident = cp.tile([128, 128], f32)
            make_identity(nc, ident[:])
            identr = cp.tile([128, 128], f32r)
            nc.sync.dma_start(out=identr[:], in_=ident[:].bitcast(f32r))

            wq_sb = cp.tile([128, D], f32r)
            nc.sync.dma_start(
                out=wq_sb[:].rearrange("p (k c) -> p k c", k=8),
                in_=Wq.rearrange("(k p) c -> p k c", p=128).bitcast(f32r))
            wk_sb = cp.tile([128, D], f32r)
            nc.sync.dma_start(
                out=wk_sb[:].rearrange("p (k c) -> p k c", k=8),
                in_=Wk.rearrange("(k p) c -> p k c", p=128).bitcast(f32r))
            bq_sb = cp.tile([CG, 1], f32)
            nc.sync.dma_start(out=bq_sb[:], in_=bq[:])
            bk_sb = cp.tile([CG, 1], f32)
            nc.sync.dma_start(out=bk_sb[:], in_=bk[:])
            mask_sb = cp.tile([128, NTILE], f32)
            nc.sync.dma_start(out=mask_sb[:], in_=maskb[:])

            # timing-only: repeat the whole body inside the NEFF
            import contextlib
            loop_ctx = (tc.For_i(0, loop_n, 1) if loop_n > 1
                        else contextlib.nullcontext())
            loop_ctx.__enter__()

            # ---- phase 1 unit: one 256-row half-block of Q or K ----
            # head h -> tile index h//2, partition offset (h%2)*32
            proj = {}  # (name, part, blk) -> [64, 512] AP (written half-wise)

            def phase1_half(X, w_sb, b_sb, nm, hb, copy_eng):
                blk, half = hb // 2, hb % 2
                r = hb * 256
                ld = sb.tile([128, 2 * D], f32r, tag="ld", bufs=3)
                nc.sync.dma_start(
                    out=ld[:].rearrange("p (s d) -> p s d", s=2),
                    in_=X[r:r + 256, :]
                    .rearrange("(s p) d -> p s d", p=128).bitcast(f32r))
                qts = sb.tile([128, 2 * D], f32r, tag="qts", bufs=3)
                for kk in range(4):     # pairs of k-chunks
                    pt = ps.tile([128, 512], f32r, tag="psB", bufs=4)
                    for dk in range(2):
                        k = kk * 2 + dk
                        for s in range(2):
                            nc.tensor.transpose(
                                pt[:, dk * 256 + s * 128: dk * 256 + (s + 1) * 128],
                                ld[:, s * D + k * 128: s * D + (k + 1) * 128],
                                identr[:])
                    use_act = (copy_eng == "act" or
                               (copy_eng == "mix" and kk % 2 == 0))
                    if use_act:
                        nc.scalar.copy(qts[:, kk * 512:(kk + 1) * 512], pt[:])
                    else:
                        nc.vector.tensor_copy(qts[:, kk * 512:(kk + 1) * 512],
                                              pt[:])
                pq = ps.tile([128, 256], f32, tag="psB", bufs=4)
                for k in range(8):
                    nc.tensor.matmul(
                        pq[:], w_sb[:, k * CG:(k + 1) * CG],
                        qts[:, k * 256:(k + 1) * 256],
                        start=(k == 0), stop=(k == 7))
                if (nm, 0, blk) not in proj:
                    proj[(nm, 0, blk)] = sb.tile([64, 512], f32r,
                                                 tag=f"{nm}a{blk}",
                                                 name=f"{nm}a{blk}")
                    proj[(nm, 1, blk)] = sb.tile([64, 512], f32r,
                                                 tag=f"{nm}b{blk}",
                                                 name=f"{nm}b{blk}")
                c0 = half * 256
                nc.vector.tensor_scalar_add(
                    proj[(nm, 0, blk)][:, c0:c0 + 256], pq[0:64, :], b_sb[0:64, :])
                nc.vector.tensor_scalar_add(
                    proj[(nm, 1, blk)][:, c0:c0 + 256], pq[64:128, :],
                    b_sb[64:128, :])

            # prologue order: the first exp only needs K block 0 + Q blocks
            # 0-1, so emit those first; remaining K blocks stream behind.
            for hb in range(2):
                phase1_half(K, wk_sb, bk_sb, "k", hb, "dve")
            for hb in range(4):
                phase1_half(Q, wq_sb, bq_sb, "q", hb, "dve")
            for hb in range(2, 8):
                phase1_half(K, wk_sb, bk_sb, "k", hb, "dve")
            deferred = [lambda hb=hb: phase1_half(Q, wq_sb, bq_sb, "q", hb,
                                                  "dve")
                        for hb in range(4, 8)]

            # ---- V: load fp32 per tile, cast to bf16 with ones cols ----
            # (emitted after the prologue: only needed once PV starts)
            vext = cp.tile([128, NTILE * 4 * VEXT], bf16)
            vext4 = vext[:].rearrange("p (t h c) -> p t h c", t=NTILE, h=4)
            nc.vector.memset(vext4[:, :, :, HV:VEXT], 1.0)
            for t in range(NTILE):
                v32 = sb.tile([128, VG], f32, tag="v32", bufs=4)
                nc.sync.dma_start(
                    out=v32[:], in_=V[t * 128:(t + 1) * 128, :])
                nc.vector.tensor_copy(
                    vext4[:, t, :, 0:HV],
                    v32[:].rearrange("p (h c) -> p h c", h=4))

            # ---- phase 2: software-pipelined attention ----
            chunks = [(tqc, h) for tqc in range(2) for h in range(4)]

            def pv_group(exps, h, tqc, j):
                po = ps.tile([128, VEXT], f32, tag="psB", bufs=4)
                for i in range(NTILE):
                    nc.tensor.matmul(
                        po[:], exps[i][:, j * 128:(j + 1) * 128],
                        vext[:, i * 4 * VEXT + h * VEXT:
                             i * 4 * VEXT + (h + 1) * VEXT],
                        start=(i == 0), stop=(i == NTILE - 1))
                rc = sb.tile([128, 1], f32, tag="rc", bufs=4)
                nc.vector.reciprocal(rc[:], po[:, HV:VEXT])
                ot = sb.tile([128, HV], f32, tag="ot", bufs=4)
                nc.vector.tensor_scalar_mul(ot[:], po[:, 0:HV], rc[:])
                row = (tqc * 8 + j) * 128
                nc.sync.dma_start(
                    out=O[row:row + 128, h * HV:(h + 1) * HV], in_=ot[:])

            prev = None  # (exps, h, tqc) awaiting PV
            for tqc, h in chunks:
                part = h // 2
                r0 = (h % 2) * HEAD_QK
                r1 = r0 + HEAD_QK
                qblks = (proj[("q", part, tqc * 2)],
                         proj[("q", part, tqc * 2 + 1)])
                exps = []
                for i in range(NTILE):
                    kblk = proj[("k", part, i // 4)]
                    lhs = kblk[r0:r1, (i % 4) * 128:(i % 4 + 1) * 128]
                    pss = ps.tile([128, 1024], f32, tag="psA", bufs=2)
                    nc.tensor.matmul(pss[:, 0:512], lhs, qblks[0][r0:r1, :],
                                     start=True, stop=True)
                    nc.tensor.matmul(pss[:, 512:1024], lhs, qblks[1][r0:r1, :],
                                     start=True, stop=True)
                    ex = sb.tile([128, 1024], bf16, tag="ex", bufs=33)
                    nc.scalar.activation(
                        ex[:], pss[:], mybir.ActivationFunctionType.Exp,
                        bias=mask_sb[:, i:i + 1], scale=SCALE)
                    exps.append(ex)
                    # interleave: PV of the previous chunk / deferred phase 1
                    if prev is not None and i % 2 == 1:
                        pv_group(prev[0], prev[1], prev[2], (i - 1) // 2)
                    elif prev is None and deferred and i % 4 == 3:
                        deferred.pop(0)()
                prev = (exps, h, tqc)
            for j in range(8):
                pv_group(prev[0], prev[1], prev[2], j)

            loop_ctx.__exit__(None, None, None)

    nc.compile()
    return nc


def _get_nc():
    if "nc" not in _cache:
        _cache["nc"] = _build()
    return _cache["nc"]


def kernel(Q, K, V, Wq, bq, Wk, bk, key_padding_mask):
    Q = np.asarray(Q, dtype=np.float32)
    K = np.asarray(K, dtype=np.float32)
    V = np.asarray(V, dtype=np.float32)
    Wq = np.asarray(Wq, dtype=np.float32)
    Wk = np.asarray(Wk, dtype=np.float32)
    bq = np.asarray(bq, dtype=np.float32)
    bk = np.asarray(bk, dtype=np.float32)
    mask = np.asarray(key_padding_mask)

    nc = _get_nc()

    in_maps = []
    for c in range(8):
        b, g = c // 2, c % 2
        mb = np.where(mask[b], np.float32(-1e30), np.float32(0.0)).astype(np.float32)
        in_maps.append({
            "Q": np.ascontiguousarray(Q[b]),
            "K": np.ascontiguousarray(K[b]),
            "V": np.ascontiguousarray(V[b, :, VG * g:VG * (g + 1)]),
            "Wq": np.ascontiguousarray(Wq[:, CG * g:CG * (g + 1)]),
            "Wk": np.ascontiguousarray(Wk[:, CG * g:CG * (g + 1)]),
            "bq": np.ascontiguousarray(bq[CG * g:CG * (g + 1)].reshape(CG, 1)),
            "bk": np.ascontiguousarray(bk[CG * g:CG * (g + 1)].reshape(CG, 1)),
            "maskb": np.ascontiguousarray(mb.reshape(NTILE, 128).T),
        })

    res = run_bass_kernel_spmd(nc, in_maps, core_ids=list(range(8)))

    out = np.empty((B, T, D), dtype=np.float32)
    for c in range(8):
        b, g = c // 2, c % 2
        out[b, :, VG * g:VG * (g + 1)] = res.results[c]["O"]
    return out



# revision 2
# speedup vs baseline: 3.1213x; 3.1213x over previous
"""Trainium2 Bass kernel for LowDimQKMultiHeadAttention.

Problem shapes (hardcoded): B=4, Tq=Tk=2048, D=1024, H=8 heads,
QK_DIM=256 (32 per head), head_v=128, fp32 I/O.

Sharding over 8 NeuronCores: core c handles batch b=c//2 and
tq/tk-half s=c%2 (rows [1024*s, 1024*s+1024) of the sequence). Every
input byte is shipped to exactly one core (no host-side duplication):
core c receives its Q-half, K-half and V-half plus the (small,
replicated) projection weights, all packed into ONE fp16 DRAM
parameter. On device, each core projects its own q/k halves; the
projected kT-half and the raw V-half are exchanged within the
2-core pair via a single pairwise AllGather (cores 2b/2b+1 share HBM,
so this is cheap), after which each core runs full attention for its
tq-half over the full Tk and writes its half of the output rows.

Transport optimizations vs the naive run_bass_kernel_spmd path:
  - fp16 wire format for Q/K/V and the output (gate is 2e-2 rel err;
    fp16 rounding contributes ~5e-4).
  - one packed input parameter -> one sharded device_put per call
    instead of nine.
  - the jitted executable is built once and cached at module level
    (the stock path re-traces and re-runs the BIR compile per call).
  - the donated pre-zeroed output operand is the previous call's
    output buffer (the kernel writes every element, so contents are
    irrelevant); only the first call ships a zeros array.

Per-core device algorithm:
  1. Project own k-half then q-half (256-row half-blocks: DMA, PE
     transpose of fp16 128x128 tiles, PSUM->SBUF stage, matmul with
     Wq/Wk stationary fp16, bias-add on the PSUM->SBUF copy).
     kT goes to a DRAM bounce tile; qT stays in SBUF.
  2. Pairwise AllGather of [kT-half | V-half] (2.5MB) while the PE
     keeps projecting q. Unpack: kT full [4x(64,2048)] SBUF tiles,
     V full into the [V | ones] extended fp16 tile (129 cols/head).
  3. Attention per head (8 chunks of tq=1024), software-pipelined as
     in the baseline: scoresT[tk=128, tq] fp32 PSUM via K=32 matmuls,
     one ACT exp per [128,1024] tile (fp16 out, fused 1/sqrt(32)
     scale + additive key-padding-mask bias), PV with fused softmax
     denominator (ones column), DVE reciprocal + scale, DMA out fp16.

NaN-scrub from the reference is skipped (inputs are finite, scores
cannot be NaN). Key padding mask is applied as an additive -60000
bias (exp underflows to 0 in fp32).
"""

import math

import numpy as np

import concourse.bacc as bacc
import concourse.mybir as mybir
import concourse.tile as tile
from concourse import bass2jax
from concourse.masks import make_identity

dt = mybir.dt

B = 4
T = 2048          # Tq == Tk
D = 1024
H = 8
HEAD_QK = 32
HV = 128          # head_v
TH = 1024         # rows per core (tq/tk half)
NTILE = 16        # 128-row tk tiles of T
SCALE = 1.0 / math.sqrt(HEAD_QK)
VEXT = HV + 1     # V cols + ones column per head

# packed input layout (rows of 512 fp16 per core)
R_Q = 0           # 2048 rows: Q-half (1024 x 1024)
R_K = 2048        # 2048 rows: K-half
R_V = 4096        # 2048 rows: V-half
R_WQ = 6144       # 512 rows: Wq packed [p=128][k=8][c=256]
R_WK = 6656       # 512 rows: Wk packed
R_BIAS = 7168     # 1 row: [bq(0:128), bq(128:256), bk(0:128), bk(128:256)] p-major
R_MASK = 7169     # 4 rows: mask bias [128,16] p-major
NR = 7173         # rows per core

_cache = {}


def _build():
    nc = bacc.Bacc("TRN2", target_bir_lowering=False, debug=False, num_devices=8)

    X = nc.declare_dram_parameter("X", [NR, 512], dt.float16, isOutput=False)
    O = nc.declare_dram_parameter("O", [2 * TH, 512], dt.float16, isOutput=True)

    f32, f16 = dt.float32, dt.float16
    Xq = X[R_Q:R_K, :].rearrange("(r s) c -> r (s c)", s=2)        # [1024, 1024]
    Xk = X[R_K:R_V, :].rearrange("(r s) c -> r (s c)", s=2)
    Ov = O[:].rearrange("(q s) c -> q (s c)", s=2)                 # [1024, 1024]

    with tile.TileContext(nc) as tc:
        with tc.tile_pool(name="consts", bufs=1) as cp, \
             tc.tile_pool(name="sb", bufs=1) as sb, \
             tc.tile_pool(name="dram", bufs=1, space="DRAM") as dram, \
             tc.tile_pool(name="ps", bufs=1, space="PSUM") as ps:
            # ---- constants ----
            identf = cp.tile([128, 128], f32)
            make_identity(nc, identf[:])
            ident = cp.tile([128, 128], f16)
            nc.vector.tensor_copy(ident[:], identf[:])

            wq_sb = cp.tile([128, 2 * D], f16)
            nc.sync.dma_start(
                out=wq_sb[:].rearrange("p (a c) -> p a c", a=4),
                in_=X[R_WQ:R_WQ + 512, :].rearrange("(p a) c -> p a c", p=128))
            wk_sb = cp.tile([128, 2 * D], f16)
            nc.sync.dma_start(
                out=wk_sb[:].rearrange("p (a c) -> p a c", a=4),
                in_=X[R_WK:R_WK + 512, :].rearrange("(p a) c -> p a c", p=128))

            bias16 = cp.tile([128, 4], f16)
            nc.sync.dma_start(
                out=bias16[:],
                in_=X[R_BIAS:R_BIAS + 1, :].rearrange("a (p j) -> (a p) j", p=128))
            bias_sb = cp.tile([128, 4], f32)
            nc.vector.tensor_copy(bias_sb[:], bias16[:])

            mask16 = cp.tile([128, NTILE], f16)
            nc.sync.dma_start(
                out=mask16[:],
                in_=X[R_MASK:R_MASK + 4, :].rearrange("a (p t) -> (a p) t", p=32))
            mask_sb = cp.tile([128, NTILE], f32)
            nc.vector.tensor_copy(mask_sb[:], mask16[:])

            # ---- collective bounce tiles ----
            # per-core contribution: kT (256x1024 as 512 rows) | V-half (2048 rows)
            cc_in = dram.tile([2560, 512], f16)
            cc_out = dram.tile([5120, 512], f16)
            cc_in_kt = cc_in[0:512, :].rearrange("(p a) c -> p (a c)", a=2)

            # ---- phase 1: project own halves ----
            # One 256-row half-block of Xq/Xk -> qT/kT [256, 256] (+bias).
            # proj q: writes SBUF tiles qt[j] [64, 1024] (head pair j).
            # proj k: writes SBUF staging then DMA into cc_in.
            qt = [cp.tile([64, TH], f16, name=f"qt{j}") for j in range(4)]

            def phase1_half(Xs, w_sb, bcol, hb, is_q):
                ld = sb.tile([128, 2 * D], f16, tag="ld", bufs=3)
                nc.sync.dma_start(
                    out=ld[:].rearrange("p (s d) -> p s d", s=2),
                    in_=Xs[hb * 256:(hb + 1) * 256, :]
                    .rearrange("(s p) d -> p s d", p=128))
                xt = sb.tile([128, 2 * D], f16, tag="xt", bufs=3)
                for kk in range(4):     # pairs of d-chunks
                    pt = ps.tile([128, 512], f16, tag="psB", bufs=4)
                    for dk in range(2):
                        k = kk * 2 + dk
                        for s in range(2):
                            nc.tensor.transpose(
                                pt[:, dk * 256 + s * 128: dk * 256 + (s + 1) * 128],
                                ld[:, s * D + k * 128: s * D + (k + 1) * 128],
                                ident[:])
                    nc.vector.tensor_copy(xt[:, kk * 512:(kk + 1) * 512], pt[:])
                for half in range(2):   # qk cols 0-127 / 128-255
                    pq = ps.tile([128, 256], f32, tag="psB", bufs=4)
                    for k in range(8):
                        nc.tensor.matmul(
                            pq[:], w_sb[:, k * 256 + half * 128:
                                        k * 256 + (half + 1) * 128],
                            xt[:, k * 256:(k + 1) * 256],
                            start=(k == 0), stop=(k == 7))
                    if is_q:
                        c0 = hb * 256
                        nc.vector.tensor_scalar_add(
                            qt[half * 2][:, c0:c0 + 256], pq[0:64, :],
                            bias_sb[0:64, bcol + half:bcol + half + 1])
                        nc.vector.tensor_scalar_add(
                            qt[half * 2 + 1][:, c0:c0 + 256], pq[64:128, :],
                            bias_sb[64:128, bcol + half:bcol + half + 1])
                    else:
                        kq = sb.tile([128, 256], f16, tag="kst", bufs=3)
                        nc.vector.tensor_scalar_add(
                            kq[:], pq[:],
                            bias_sb[:, bcol + half:bcol + half + 1])
                        nc.sync.dma_start(
                            out=cc_in_kt[half * 128:(half + 1) * 128,
                                         hb * 256:(hb + 1) * 256],
                            in_=kq[:])

            # K first: the collective input must be complete early.
            for hb in range(4):
                phase1_half(Xk, wk_sb, 2, hb, False)
            # V-half straight into the collective input (DRAM -> DRAM).
            nc.sync.dma_start(out=cc_in[512:2560, :], in_=X[R_V:R_V + 2048, :])

            nc.gpsimd.collective_compute(
                "AllGather",
                mybir.AluOpType.bypass,
                replica_groups=[[0, 1], [2, 3], [4, 5], [6, 7]],
                ins=[cc_in.opt()],
                outs=[cc_out.opt()],
            )

            # Q projection overlaps the collective.
            for hb in range(4):
                phase1_half(Xq, wq_sb, 0, hb, True)

            # ---- unpack gathered kT / V ----
            kt = [cp.tile([64, T], f16, name=f"kt{j}") for j in range(4)]
            for blob in range(2):
                src = cc_out[blob * 2560: blob * 2560 + 512, :] \
                    .rearrange("(p a) c -> p (a c)", a=2)   # [256, 1024]
                for j in range(4):
                    nc.sync.dma_start(
                        out=kt[j][:, blob * TH:(blob + 1) * TH],
                        in_=src[j * 64:(j + 1) * 64, :])

            vext = cp.tile([128, NTILE * H * VEXT], f16)
            vext4 = vext[:].rearrange("p (t h c) -> p t h c", t=NTILE, h=H)
            nc.vector.memset(vext4[:, :, :, HV:VEXT], 1.0)
            for t in range(NTILE):
                blob, tl = t // 8, t % 8
                vt = sb.tile([128, D], f16, tag="vt", bufs=4)
                nc.sync.dma_start(
                    out=vt[:].rearrange("p (s c) -> p s c", s=2),
                    in_=cc_out[blob * 2560 + 512 + tl * 256:
                               blob * 2560 + 512 + (tl + 1) * 256, :]
                    .rearrange("(p s) c -> p s c", p=128))
                nc.vector.tensor_copy(
                    vext4[:, t, :, 0:HV],
                    vt[:].rearrange("p (h c) -> p h c", h=H))

            # ---- phase 2: software-pipelined attention over 8 heads ----
            def pv_group(exps, h, j):
                po = ps.tile([128, VEXT], f32, tag="psB", bufs=4)
                for i in range(NTILE):
                    nc.tensor.matmul(
                        po[:], exps[i][:, j * 128:(j + 1) * 128],
                        vext[:, i * H * VEXT + h * VEXT:
                             i * H * VEXT + (h + 1) * VEXT],
                        start=(i == 0), stop=(i == NTILE - 1))
                rc = sb.tile([128, 1], f32, tag="rc", bufs=4)
                nc.vector.reciprocal(rc[:], po[:, HV:VEXT])
                ot = sb.tile([128, HV], f16, tag="ot", bufs=4)
                nc.vector.tensor_scalar_mul(ot[:], po[:, 0:HV], rc[:])
                nc.sync.dma_start(
                    out=Ov[j * 128:(j + 1) * 128, h * HV:(h + 1) * HV],
                    in_=ot[:])

            prev = None  # (exps, h) awaiting PV
            for h in range(H):
                part, r0 = h // 2, (h % 2) * HEAD_QK
                r1 = r0 + HEAD_QK
                exps = []
                for i in range(NTILE):
                    lhs = kt[part][r0:r1, i * 128:(i + 1) * 128]
                    pss = ps.tile([128, TH], f32, tag="psA", bufs=2)
                    nc.tensor.matmul(pss[:, 0:512], lhs, qt[part][r0:r1, 0:512],
                                     start=True, stop=True)
                    nc.tensor.matmul(pss[:, 512:1024], lhs, qt[part][r0:r1, 512:1024],
                                     start=True, stop=True)
                    ex = sb.tile([128, TH], f16, tag="ex", bufs=33)
                    nc.scalar.activation(
                        ex[:], pss[:], mybir.ActivationFunctionType.Exp,
                        bias=mask_sb[:, i:i + 1], scale=SCALE)
                    exps.append(ex)
                    if prev is not None and i % 2 == 1:
                        pv_group(prev[0], prev[1], (i - 1) // 2)
                prev = (exps, h)
            for j in range(8):
                pv_group(prev[0], prev[1], j)

    nc.compile()
    return nc


def _make_runner(nc, n_cores=8):
    import jax
    from jax.sharding import Mesh, NamedSharding, PartitionSpec
    from jax.experimental.shard_map import shard_map

    bass2jax.install_neuronx_cc_hook()
    partition_name = nc.partition_id_tensor.name if nc.partition_id_tensor else None
    in_names, out_names, out_avals = [], [], []
    for alloc in nc.m.functions[0].allocations:
        if not isinstance(alloc, mybir.MemoryLocationSet):
            continue
        name = alloc.memorylocations[0].name
        if alloc.kind == "ExternalInput":
            if name != partition_name:
                in_names.append(name)
        elif alloc.kind == "ExternalOutput":
            out_avals.append(jax.core.ShapedArray(
                tuple(alloc.tensor_shape), mybir.dt.np(alloc.dtype)))
            out_names.append(name)
    n_params = len(in_names)
    n_outs = len(out_names)
    in_names = in_names + out_names
    if partition_name is not None:
        in_names.append(partition_name)

    def _body(*args):
        operands = list(args)
        if partition_name is not None:
            operands.append(bass2jax.partition_id_tensor())
        outs = bass2jax._bass_exec_p.bind(
            *operands,
            out_avals=tuple(out_avals),
            in_names=tuple(in_names),
            out_names=tuple(out_names),
            lowering_input_output_aliases=(),
            sim_require_finite=True,
            sim_require_nnan=True,
            nc=nc,
        )
        return tuple(outs)

    devices = jax.devices()[:n_cores]
    mesh = Mesh(np.asarray(devices), ("core",))
    fn = jax.jit(
        shard_map(_body, mesh=mesh,
                  in_specs=(PartitionSpec("core"),) * (n_params + n_outs),
                  out_specs=(PartitionSpec("core"),) * n_outs,
                  check_rep=False),
        donate_argnums=tuple(range(n_params, n_params + n_outs)),
        keep_unused=True,
    )
    sharding = NamedSharding(mesh, PartitionSpec("core"))
    return fn, sharding


def _get_runner():
    if "runner" not in _cache:
        nc = _build()
        fn, sharding = _make_runner(nc)
        _cache["runner"] = (fn, sharding)
        _cache["G"] = np.empty((8 * NR, 512), np.float16)
        _cache["carry"] = np.zeros((8 * 2 * TH, 512), np.float16)
    return _cache["runner"]


def kernel(Q, K, V, Wq, bq, Wk, bk, key_padding_mask):
    import jax

    fn, sharding = _get_runner()

    Q = np.asarray(Q, dtype=np.float32).reshape(8, TH, D)
    K = np.asarray(K, dtype=np.float32).reshape(8, TH, D)
    V = np.asarray(V, dtype=np.float32).reshape(8, TH, D)
    Wq = np.asarray(Wq, dtype=np.float32)
    Wk = np.asarray(Wk, dtype=np.float32)
    bq = np.asarray(bq, dtype=np.float32)
    bk = np.asarray(bk, dtype=np.float32)
    mask = np.asarray(key_padding_mask)

    wq_pk = Wq.reshape(8, 128, 256).transpose(1, 0, 2).reshape(512, 512)
    wk_pk = Wk.reshape(8, 128, 256).transpose(1, 0, 2).reshape(512, 512)
    bias_row = np.stack(
        [bq[0:128], bq[128:256], bk[0:128], bk[128:256]], axis=1).reshape(512)
    maskb = np.where(mask, np.float32(-60000.0), np.float32(0.0))
    # [b][128,16] p-major -> 4 rows of 512
    mask_rows = maskb.reshape(B, NTILE, 128).transpose(0, 2, 1).reshape(B, 4, 512)

    G = _cache["G"]
    Gc = G.reshape(8, NR, 512)
    for c in range(8):
        Gc[c, R_Q:R_K] = Q[c].reshape(2048, 512)
        Gc[c, R_K:R_V] = K[c].reshape(2048, 512)
        Gc[c, R_V:R_WQ] = V[c].reshape(2048, 512)
        Gc[c, R_WQ:R_WK] = wq_pk
        Gc[c, R_WK:R_BIAS] = wk_pk
        Gc[c, R_BIAS] = bias_row
        Gc[c, R_MASK:NR] = mask_rows[c // 2]

    x_dev = jax.device_put(G, sharding)
    out, = fn(x_dev, _cache["carry"])
    _cache["carry"] = out
    res = np.asarray(out)
    return res.reshape(B, T, D).astype(np.float32)


# revision 4
# speedup vs baseline: 3.1291x; 1.0025x over previous
"""Trainium2 Bass kernel for LowDimQKMultiHeadAttention.

Problem shapes (hardcoded): B=4, Tq=Tk=2048, D=1024, H=8 heads,
QK_DIM=256 (32 per head), head_v=128, fp32 I/O.

Sharding over 8 NeuronCores: core c handles batch b=c//2 and
tq/tk-half s=c%2 (rows [1024*s, 1024*s+1024) of the sequence). Every
input byte is shipped to exactly one core (no host-side duplication):
core c receives its Q-half, K-half and V-half plus the (small,
replicated) projection weights, all packed into ONE fp16 DRAM
parameter. On device, each core projects its own q/k halves; the
projected kT-half and the raw V-half are exchanged within the
2-core pair via a single pairwise AllGather (cores 2b/2b+1 share HBM,
so this is cheap), after which each core runs full attention for its
tq-half over the full Tk and writes its half of the output rows.

Transport optimizations vs the naive run_bass_kernel_spmd path:
  - fp16 wire format for Q/K/V and the output (gate is 2e-2 rel err;
    fp16 rounding contributes ~5e-4).
  - one packed input parameter -> one sharded device_put per call
    instead of nine.
  - the jitted executable is built once and cached at module level
    (the stock path re-traces and re-runs the BIR compile per call).
  - the donated pre-zeroed output operand is the previous call's
    output buffer (the kernel writes every element, so contents are
    irrelevant); only the first call ships a zeros array.

Per-core device algorithm:
  1. Project own k-half then q-half (256-row half-blocks: DMA, PE
     transpose of fp16 128x128 tiles, PSUM->SBUF stage, matmul with
     Wq/Wk stationary fp16, bias-add on the PSUM->SBUF copy).
     kT goes to a DRAM bounce tile; qT stays in SBUF.
  2. Pairwise AllGather of [kT-half | V-half] (2.5MB) while the PE
     keeps projecting q. Unpack: kT full [4x(64,2048)] SBUF tiles,
     V full into the [V | ones] extended fp16 tile (129 cols/head).
  3. Attention per head (8 chunks of tq=1024), software-pipelined as
     in the baseline: scoresT[tk=128, tq] fp32 PSUM via K=32 matmuls,
     one ACT exp per [128,1024] tile (fp16 out, fused 1/sqrt(32)
     scale + additive key-padding-mask bias), PV with fused softmax
     denominator (ones column), DVE reciprocal + scale, DMA out fp16.

NaN-scrub from the reference is skipped (inputs are finite, scores
cannot be NaN). Key padding mask is applied as an additive -60000
bias (exp underflows to 0 in fp32).
"""

import math

import numpy as np

import concourse.bacc as bacc
import concourse.mybir as mybir
import concourse.tile as tile
from concourse import bass2jax
from concourse.masks import make_identity

dt = mybir.dt

B = 4
T = 2048          # Tq == Tk
D = 1024
H = 8
HEAD_QK = 32
HV = 128          # head_v
TH = 1024         # rows per core (tq/tk half)
NTILE = 16        # 128-row tk tiles of T
SCALE = 1.0 / math.sqrt(HEAD_QK)
VEXT = HV + 1     # V cols + ones column per head

# packed input layout (rows of 512 fp16 per core)
R_Q = 0           # 2048 rows: Q-half (1024 x 1024)
R_K = 2048        # 2048 rows: K-half
R_V = 4096        # 2048 rows: V-half
R_WQ = 6144       # 512 rows: Wq packed [p=128][k=8][c=256]
R_WK = 6656       # 512 rows: Wk packed
R_BIAS = 7168     # 1 row: [bq(0:128), bq(128:256), bk(0:128), bk(128:256)] p-major
R_MASK = 7169     # 4 rows: mask bias [128,16] p-major
NR = 7173         # rows per core

_cache = {}


def _build():
    nc = bacc.Bacc("TRN2", target_bir_lowering=False, debug=False, num_devices=8)

    X = nc.declare_dram_parameter("X", [NR, 512], dt.bfloat16, isOutput=False)
    O = nc.declare_dram_parameter("O", [2 * TH, 512], dt.float16, isOutput=True)

    f32, f16, bf16 = dt.float32, dt.float16, dt.bfloat16
    Xq = X[R_Q:R_K, :].rearrange("(r s) c -> r (s c)", s=2)        # [1024, 1024]
    Xk = X[R_K:R_V, :].rearrange("(r s) c -> r (s c)", s=2)
    Ov = O[:].rearrange("(q s) c -> q (s c)", s=2)                 # [1024, 1024]

    with tile.TileContext(nc) as tc:
        with tc.tile_pool(name="consts", bufs=1) as cp, \
             tc.tile_pool(name="sb", bufs=1) as sb, \
             tc.tile_pool(name="dram", bufs=1, space="DRAM") as dram, \
             tc.tile_pool(name="ps", bufs=1, space="PSUM") as ps:
            # ---- constants ----
            identf = cp.tile([128, 128], f32)
            make_identity(nc, identf[:])
            ident = cp.tile([128, 128], bf16)
            nc.vector.tensor_copy(ident[:], identf[:])

            wq_sb = cp.tile([128, 2 * D], bf16)
            nc.sync.dma_start(
                out=wq_sb[:].rearrange("p (a c) -> p a c", a=4),
                in_=X[R_WQ:R_WQ + 512, :].rearrange("(p a) c -> p a c", p=128))
            wk_sb = cp.tile([128, 2 * D], bf16)
            nc.sync.dma_start(
                out=wk_sb[:].rearrange("p (a c) -> p a c", a=4),
                in_=X[R_WK:R_WK + 512, :].rearrange("(p a) c -> p a c", p=128))

            bias16 = cp.tile([128, 4], bf16)
            nc.sync.dma_start(
                out=bias16[:],
                in_=X[R_BIAS:R_BIAS + 1, :].rearrange("a (p j) -> (a p) j", p=128))
            bias_sb = cp.tile([128, 4], f32)
            nc.vector.tensor_copy(bias_sb[:], bias16[:])

            mask16 = cp.tile([128, NTILE], bf16)
            nc.sync.dma_start(
                out=mask16[:],
                in_=X[R_MASK:R_MASK + 4, :].rearrange("a (p t) -> (a p) t", p=32))
            mask_sb = cp.tile([128, NTILE], f32)
            nc.vector.tensor_copy(mask_sb[:], mask16[:])

            # ---- collective bounce tiles ----
            # per-core contribution: kT (256x1024 as 512 rows) | V-half (2048 rows)
            cc_in = dram.tile([2560, 512], bf16)
            cc_out = dram.tile([5120, 512], bf16)
            cc_in_kt = cc_in[0:512, :].rearrange("(p a) c -> p (a c)", a=2)

            # ---- phase 1: project own halves ----
            # One 256-row half-block of Xq/Xk -> qT/kT [256, 256] (+bias).
            # proj q: writes SBUF tiles qt[j] [64, 1024] (head pair j).
            # proj k: writes SBUF staging then DMA into cc_in.
            qt = [cp.tile([64, TH], bf16, name=f"qt{j}") for j in range(4)]

            def phase1_half(Xs, w_sb, bcol, hb, is_q):
                ld = sb.tile([128, 2 * D], bf16, tag="ld", bufs=3)
                nc.sync.dma_start(
                    out=ld[:].rearrange("p (s d) -> p s d", s=2),
                    in_=Xs[hb * 256:(hb + 1) * 256, :]
                    .rearrange("(s p) d -> p s d", p=128))
                xt = sb.tile([128, 2 * D], bf16, tag="xt", bufs=3)
                for kk in range(4):     # pairs of d-chunks
                    pt = ps.tile([128, 512], bf16, tag="psB", bufs=4)
                    for dk in range(2):
                        k = kk * 2 + dk
                        for s in range(2):
                            nc.tensor.transpose(
                                pt[:, dk * 256 + s * 128: dk * 256 + (s + 1) * 128],
                                ld[:, s * D + k * 128: s * D + (k + 1) * 128],
                                ident[:])
                    nc.vector.tensor_copy(xt[:, kk * 512:(kk + 1) * 512], pt[:])
                for half in range(2):   # qk cols 0-127 / 128-255
                    pq = ps.tile([128, 256], f32, tag="psB", bufs=4)
                    for k in range(8):
                        nc.tensor.matmul(
                            pq[:], w_sb[:, k * 256 + half * 128:
                                        k * 256 + (half + 1) * 128],
                            xt[:, k * 256:(k + 1) * 256],
                            start=(k == 0), stop=(k == 7))
                    if is_q:
                        c0 = hb * 256
                        nc.vector.tensor_scalar_add(
                            qt[half * 2][:, c0:c0 + 256], pq[0:64, :],
                            bias_sb[0:64, bcol + half:bcol + half + 1])
                        nc.vector.tensor_scalar_add(
                            qt[half * 2 + 1][:, c0:c0 + 256], pq[64:128, :],
                            bias_sb[64:128, bcol + half:bcol + half + 1])
                    else:
                        kq = sb.tile([128, 256], bf16, tag="kst", bufs=3)
                        nc.vector.tensor_scalar_add(
                            kq[:], pq[:],
                            bias_sb[:, bcol + half:bcol + half + 1])
                        nc.sync.dma_start(
                            out=cc_in_kt[half * 128:(half + 1) * 128,
                                         hb * 256:(hb + 1) * 256],
                            in_=kq[:])

            # K first: the collective input must be complete early.
            for hb in range(4):
                phase1_half(Xk, wk_sb, 2, hb, False)
            # V-half straight into the collective input (DRAM -> DRAM).
            nc.sync.dma_start(out=cc_in[512:2560, :], in_=X[R_V:R_V + 2048, :])

            nc.gpsimd.collective_compute(
                "AllGather",
                mybir.AluOpType.bypass,
                replica_groups=[[0, 1], [2, 3], [4, 5], [6, 7]],
                ins=[cc_in.opt()],
                outs=[cc_out.opt()],
            )

            # Q projection overlaps the collective.
            for hb in range(4):
                phase1_half(Xq, wq_sb, 0, hb, True)

            # ---- unpack gathered kT / V ----
            kt = [cp.tile([64, T], bf16, name=f"kt{j}") for j in range(4)]
            for blob in range(2):
                src = cc_out[blob * 2560: blob * 2560 + 512, :] \
                    .rearrange("(p a) c -> p (a c)", a=2)   # [256, 1024]
                for j in range(4):
                    nc.sync.dma_start(
                        out=kt[j][:, blob * TH:(blob + 1) * TH],
                        in_=src[j * 64:(j + 1) * 64, :])

            vext = cp.tile([128, NTILE * H * VEXT], f16)
            vext4 = vext[:].rearrange("p (t h c) -> p t h c", t=NTILE, h=H)
            nc.vector.memset(vext4[:, :, :, HV:VEXT], 1.0)
            for t in range(NTILE):
                blob, tl = t // 8, t % 8
                vt = sb.tile([128, D], bf16, tag="vt", bufs=4)
                nc.sync.dma_start(
                    out=vt[:].rearrange("p (s c) -> p s c", s=2),
                    in_=cc_out[blob * 2560 + 512 + tl * 256:
                               blob * 2560 + 512 + (tl + 1) * 256, :]
                    .rearrange("(p s) c -> p s c", p=128))
                nc.vector.tensor_copy(
                    vext4[:, t, :, 0:HV],
                    vt[:].rearrange("p (h c) -> p h c", h=H))

            # ---- phase 2: software-pipelined attention over 8 heads ----
            def pv_group(exps, h, j):
                po = ps.tile([128, VEXT], f32, tag="psB", bufs=4)
                for i in range(NTILE):
                    nc.tensor.matmul(
                        po[:], exps[i][:, j * 128:(j + 1) * 128],
                        vext[:, i * H * VEXT + h * VEXT:
                             i * H * VEXT + (h + 1) * VEXT],
                        start=(i == 0), stop=(i == NTILE - 1))
                rc = sb.tile([128, 1], f32, tag="rc", bufs=4)
                nc.vector.reciprocal(rc[:], po[:, HV:VEXT])
                ot = sb.tile([128, HV], f16, tag="ot", bufs=4)
                nc.vector.tensor_scalar_mul(ot[:], po[:, 0:HV], rc[:])
                nc.sync.dma_start(
                    out=Ov[j * 128:(j + 1) * 128, h * HV:(h + 1) * HV],
                    in_=ot[:])

            prev = None  # (exps, h) awaiting PV
            for h in range(H):
                part, r0 = h // 2, (h % 2) * HEAD_QK
                r1 = r0 + HEAD_QK
                exps = []
                for i in range(NTILE):
                    lhs = kt[part][r0:r1, i * 128:(i + 1) * 128]
                    pss = ps.tile([128, TH], f32, tag="psA", bufs=2)
                    nc.tensor.matmul(pss[:, 0:512], lhs, qt[part][r0:r1, 0:512],
                                     start=True, stop=True)
                    nc.tensor.matmul(pss[:, 512:1024], lhs, qt[part][r0:r1, 512:1024],
                                     start=True, stop=True)
                    ex = sb.tile([128, TH], f16, tag="ex", bufs=33)
                    nc.scalar.activation(
                        ex[:], pss[:], mybir.ActivationFunctionType.Exp,
                        bias=mask_sb[:, i:i + 1], scale=SCALE)
                    exps.append(ex)
                    if prev is not None and i % 2 == 1:
                        pv_group(prev[0], prev[1], (i - 1) // 2)
                prev = (exps, h)
            for j in range(8):
                pv_group(prev[0], prev[1], j)

    nc.compile()
    return nc


def _make_runner(nc, n_cores=8):
    import jax
    from jax.sharding import Mesh, NamedSharding, PartitionSpec
    from jax.experimental.shard_map import shard_map

    bass2jax.install_neuronx_cc_hook()
    partition_name = nc.partition_id_tensor.name if nc.partition_id_tensor else None
    in_names, out_names, out_avals = [], [], []
    for alloc in nc.m.functions[0].allocations:
        if not isinstance(alloc, mybir.MemoryLocationSet):
            continue
        name = alloc.memorylocations[0].name
        if alloc.kind == "ExternalInput":
            if name != partition_name:
                in_names.append(name)
        elif alloc.kind == "ExternalOutput":
            out_avals.append(jax.core.ShapedArray(
                tuple(alloc.tensor_shape), mybir.dt.np(alloc.dtype)))
            out_names.append(name)
    n_params = len(in_names)
    n_outs = len(out_names)
    in_names = in_names + out_names
    if partition_name is not None:
        in_names.append(partition_name)

    def _body(*args):
        operands = list(args)
        if partition_name is not None:
            operands.append(bass2jax.partition_id_tensor())
        outs = bass2jax._bass_exec_p.bind(
            *operands,
            out_avals=tuple(out_avals),
            in_names=tuple(in_names),
            out_names=tuple(out_names),
            lowering_input_output_aliases=(),
            sim_require_finite=True,
            sim_require_nnan=True,
            nc=nc,
        )
        return tuple(outs)

    devices = jax.devices()[:n_cores]
    mesh = Mesh(np.asarray(devices), ("core",))
    fn = jax.jit(
        shard_map(_body, mesh=mesh,
                  in_specs=(PartitionSpec("core"),) * (n_params + n_outs),
                  out_specs=(PartitionSpec("core"),) * n_outs,
                  check_rep=False),
        donate_argnums=tuple(range(n_params, n_params + n_outs)),
        keep_unused=True,
    )
    sharding = NamedSharding(mesh, PartitionSpec("core"))
    return fn, sharding


def _get_runner():
    if "runner" not in _cache:
        nc = _build()
        fn, sharding = _make_runner(nc)
        _cache["runner"] = (fn, sharding)
        import ml_dtypes
        _cache["G"] = np.empty((8 * NR, 512), ml_dtypes.bfloat16)
        _cache["carry"] = np.zeros((8 * 2 * TH, 512), np.float16)
    return _cache["runner"]


def kernel(Q, K, V, Wq, bq, Wk, bk, key_padding_mask):
    import jax

    fn, sharding = _get_runner()

    Q = np.asarray(Q, dtype=np.float32).reshape(8, TH, D)
    K = np.asarray(K, dtype=np.float32).reshape(8, TH, D)
    V = np.asarray(V, dtype=np.float32).reshape(8, TH, D)
    Wq = np.asarray(Wq, dtype=np.float32)
    Wk = np.asarray(Wk, dtype=np.float32)
    bq = np.asarray(bq, dtype=np.float32)
    bk = np.asarray(bk, dtype=np.float32)
    mask = np.asarray(key_padding_mask)

    wq_pk = Wq.reshape(8, 128, 256).transpose(1, 0, 2).reshape(512, 512)
    wk_pk = Wk.reshape(8, 128, 256).transpose(1, 0, 2).reshape(512, 512)
    bias_row = np.stack(
        [bq[0:128], bq[128:256], bk[0:128], bk[128:256]], axis=1).reshape(512)
    maskb = np.where(mask, np.float32(-60000.0), np.float32(0.0))
    # [b][128,16] p-major -> 4 rows of 512
    mask_rows = maskb.reshape(B, NTILE, 128).transpose(0, 2, 1).reshape(B, 4, 512)

    G = _cache["G"]
    Gc = G.reshape(8, NR, 512)
    for c in range(8):
        Gc[c, R_Q:R_K] = Q[c].reshape(2048, 512)
        Gc[c, R_K:R_V] = K[c].reshape(2048, 512)
        Gc[c, R_V:R_WQ] = V[c].reshape(2048, 512)
        Gc[c, R_WQ:R_WK] = wq_pk
        Gc[c, R_WK:R_BIAS] = wk_pk
        Gc[c, R_BIAS] = bias_row
        Gc[c, R_MASK:NR] = mask_rows[c // 2]

    x_dev = jax.device_put(G, sharding)
    out, = fn(x_dev, _cache["carry"])
    _cache["carry"] = out
    res = np.asarray(out)
    return res.reshape(B, T, D).astype(np.float32)
